# revision 8
# baseline (speedup 1.0000x reference)
"""Dempster-Shafer sequential fusion kernel for Trainium2 (Bass/Tile).

Reference computation (per batch row b):
    m = x[b, 0, :]
    for k in 1..D-1:
        alpha = x[b, k, :] + x[b, k, w]          # w = C-1 (omega channel)
        m     = m * alpha + m[w] * x[b, k, :]
        m     = m / sum(m)                        # renormalize每step
    out[b] = m

Key algebra: the combine is bilinear, so per-step normalization only changes
the per-row scale -> normalize once at the end.  Tracking s = m / m[w]
(note m[w] follows m[w] <- 3*m[w]*x[b,k,w] exactly) gives the scaled
recurrence
    s_k = (u_k * alpha_k) * s_{k-1} + u_k * x_k,   u_k = 1/(3 x[b,k,w])
    s_0 = x_0 / x_0[w]
which maps 1:1 onto the DVE tensor_tensor_scan op
    state = (data0[t] * state) + data1[t]
run along a c-major free-dim sequence: for each class c, a reset position
(data0=0, data1=initial state) followed by the 127 scan steps.  Chunked
along d; chunk-to-chunk state handoff goes through the reset positions.

Sharding: pure data parallel, batch axis split across 8 NeuronCores.
"""

import numpy as np

# Problem geometry (hardcoded per the harness contract).
B, D, C = 4096, 128, 101
N_CORES = 8
BC = B // N_CORES          # batch rows per core (512)
P = 128                    # SBUF partitions = rows per group
N_GROUPS = BC // P         # 4
KS_LIST = (32, 32, 32, 31)  # chunk sizes over k = 1..D-1 (sum = 127)

_CACHED = {}


def _build_nc():
    import contextlib

    import concourse.bacc as bacc
    import concourse.tile as tile
    from concourse import mybir

    f32 = mybir.dt.float32
    # Bacc (not plain Bass): its compile() runs generate_event_semaphores,
    # which splits multi-sem waits into EventSemaphore instructions — the
    # TRN2 ISA allows at most one sync wait per regular instruction.
    nc = bacc.Bacc("TRN2", target_bir_lowering=False, debug=True)
    x = nc.declare_dram_parameter("inputs", [BC, D, C], f32, isOutput=False)
    y = nc.declare_dram_parameter("output", [BC, C], f32, isOutput=True)

    with tile.TileContext(nc) as tc, contextlib.ExitStack() as ctx:
        xpool = ctx.enter_context(tc.tile_pool(name="xin", bufs=3))
        d0pool = ctx.enter_context(tc.tile_pool(name="d0", bufs=2))
        d1pool = ctx.enter_context(tc.tile_pool(name="d1", bufs=2))
        opool = ctx.enter_context(tc.tile_pool(name="scanout", bufs=2))
        spool = ctx.enter_context(tc.tile_pool(name="small", bufs=4))

        for g in range(N_GROUPS):
            rows = slice(g * P, (g + 1) * P)

            # First source: init state s0 = x0 / x0[w].
            x0 = spool.tile([P, C], f32, tag="x0")
            nc.sync.dma_start(out=x0, in_=x[rows, 0, :])
            u0 = spool.tile([P, 1], f32, tag="u0")
            nc.vector.reciprocal(u0, x0[:, C - 1 : C])

            prev_out = None
            prev_ch = None
            k0 = 1
            for j, ks in enumerate(KS_LIST):
                ch = ks + 1  # reset col + ks scan cols per class
                xt = xpool.tile([P, ks, C], f32, tag="xt")
                nc.sync.dma_start(out=xt, in_=x[rows, k0 : k0 + ks, :])

                # u_k = 1 / (3 * x[:, k, w]) for the chunk.
                t3 = spool.tile([P, ks], f32, tag="t3")
                nc.vector.tensor_scalar_mul(
                    out=t3, in0=xt[:, :, C - 1], scalar1=3.0
                )
                uc = spool.tile([P, ks], f32, tag="uc")
                nc.vector.reciprocal(uc, t3)

                d0 = d0pool.tile([P, C, ch], f32, tag="d0")
                d1 = d1pool.tile([P, C, ch], f32, tag="d1")
                ot = opool.tile([P, C, ch], f32, tag="ot")

                # Reset columns: data0 = 0 so state restarts from data1.
                # All producers stay on the vector engine: same-engine program
                # order needs no semaphores, and the walrus codegen has a hard
                # cap on sync-wait commands per instruction.
                nc.vector.memset(d0[:, :, 0], 0.0)
                if j == 0:
                    nc.vector.tensor_scalar_mul(
                        out=d1[:, :, 0], in0=x0, scalar1=u0
                    )
                else:
                    nc.vector.tensor_copy(d1[:, :, 0], prev_out[:, :, prev_ch - 1])

                # data1 scan cols: y = x * u   ([P, C, ks], c outer / k inner)
                x_ck = xt.transpose([0, 2, 1])  # [P, C, ks] strided view
                u_b = uc.unsqueeze(1).to_broadcast([P, C, ks])
                nc.vector.tensor_mul(out=d1[:, :, 1:], in0=x_ck, in1=u_b)

                # data0 scan cols: alpha' = y + y[w]  (broadcast over c)
                yw_b = d1[:, C - 1 : C, 1:].to_broadcast([P, C, ks])
                nc.vector.tensor_add(out=d0[:, :, 1:], in0=d1[:, :, 1:], in1=yw_b)

                # The scan: state = data0*state + data1 along the free dim.
                nc.vector.tensor_tensor_scan(
                    out=ot.rearrange("p c k -> p (c k)"),
                    data0=d0.rearrange("p c k -> p (c k)"),
                    data1=d1.rearrange("p c k -> p (c k)"),
                    initial=0.0,
                    op0=mybir.AluOpType.mult,
                    op1=mybir.AluOpType.add,
                )
                prev_out, prev_ch = ot, ch
                k0 += ks

            # Final states -> normalize -> store.
            res = spool.tile([P, C], f32, tag="res")
            nc.vector.tensor_copy(res, prev_out[:, :, prev_ch - 1])
            ssum = spool.tile([P, 1], f32, tag="ssum")
            nc.vector.reduce_sum(ssum, res, axis=mybir.AxisListType.X)
            rec = spool.tile([P, 1], f32, tag="rec")
            nc.vector.reciprocal(rec, ssum)
            res2 = spool.tile([P, C], f32, tag="res2")
            nc.vector.tensor_scalar_mul(out=res2, in0=res, scalar1=rec)
            nc.sync.dma_start(out=y[rows, :], in_=res2)

    nc.compile()
    return nc


def _get_nc():
    if "nc" not in _CACHED:
        _CACHED["nc"] = _build_nc()
    return _CACHED["nc"]


def kernel(inputs: np.ndarray) -> np.ndarray:
    from concourse.bass_utils import run_bass_kernel_spmd

    inputs = np.asarray(inputs, dtype=np.float32)
    assert inputs.shape == (B, D, C), inputs.shape

    nc = _get_nc()
    in_maps = [
        {"inputs": np.ascontiguousarray(inputs[i * BC : (i + 1) * BC])}
        for i in range(N_CORES)
    ]
    out = run_bass_kernel_spmd(nc, in_maps, list(range(N_CORES)))
    return np.concatenate(
        [out.results[i]["output"] for i in range(N_CORES)], axis=0
    )
